# revision 4
# baseline (speedup 1.0000x reference)
"""CartBasisStressHead kernel for Trainium2 (8 NeuronCores, SPMD data-parallel).

Strategy
--------
Only 6 of the 9 m-rows of node_embedding are used: row 0 feeds a SiLU MLP
(per-node scalar), rows 4:9 feed a per-channel contraction (l=2 branch).
Nodes are sharded contiguously across 8 cores (graphs may straddle shard
boundaries; segment sums are linear, so host adds the partials).

Per core the device computes, in 1024-node groups:
  * scalar branch: h = silu(W1 @ x0T + b1); h = silu(W2 @ h + b2);
    s = W3 @ h  -> per-node scalars (feature-major layout, x0 transposed
    on host so the contraction dim lands on SBUF partitions).
  * l=2 branch fused with the segment sum: a 0/1 indicator matrix
    A[node, local_graph] (built on the vector engine from iota == local
    graph id) is the stationary matmul operand, so
    S[g_local, (m,c)] = sum_n A[n,g] * emb[n, (m,c)] accumulates in PSUM
    across the group's 8 node-tiles. Data streams through the PE once.

Host epilogue: bincount segment-sum of per-node scalars, scatter-add of
per-group S partials, contraction with w_l2, and the tiny (G,9)@(9,9)
change-of-basis.
"""

import sys

if "/opt/trn_rl_repo" not in sys.path:
    sys.path.insert(0, "/opt/trn_rl_repo")

import numpy as np

import concourse.bacc as bacc
import concourse.tile as tile
from concourse import mybir
from concourse import bass_utils

_S2 = 2.0 ** -0.5
_S3 = 3.0 ** -0.5
_S6 = 6.0 ** -0.5
_CG = np.array([
    [_S3, 0, 0, 0, _S3, 0, 0, 0, _S3],
    [0, 0, 0, 0, 0, _S2, 0, -_S2, 0],
    [0, 0, -_S2, 0, 0, 0, _S2, 0, 0],
    [0, _S2, 0, -_S2, 0, 0, 0, 0, 0],
    [0, 0, _S2, 0, 0, 0, _S2, 0, 0],
    [0, 0, 0, 0, 0, _S2, 0, _S2, 0],
    [-_S6, 0, 0, 0, 2 * _S6, 0, 0, 0, -_S6],
    [0, _S2, 0, _S2, 0, 0, 0, 0, 0],
    [-_S2, 0, 0, 0, 0, 0, 0, 0, _S2],
], dtype=np.float32)  # (9, 9)

N_CORES = 8
P = 128          # SBUF partitions
NG = 1024        # nodes per group (one PSUM accumulation span)
T = NG // P      # node-tiles per group
SUP = NG // 512  # 512-node supertiles per group (MLP granularity)
ML2 = 5 * P      # 640 floats of l=2 data per node

F32 = mybir.dt.float32
F32R = mybir.dt.float32r

_BUILD_CACHE = {}


def _build(n_pad, n_groups, W, use_f32r=True):
    key = (n_pad, n_groups, W, use_f32r)
    if key in _BUILD_CACHE:
        return _BUILD_CACHE[key]

    nc = bacc.Bacc("TRN2", target_bir_lowering=False, debug=False,
                   num_devices=N_CORES)

    mdt = F32R if use_f32r else F32
    x0T = nc.dram_tensor("x0T", (P, n_pad), mdt, kind="ExternalInput").ap()
    embL2 = nc.dram_tensor("embL2", (n_pad, ML2), mdt, kind="ExternalInput").ap()
    lgid = nc.dram_tensor("lgid", (P, n_groups * T), F32, kind="ExternalInput").ap()
    iota_in = nc.dram_tensor("iota_in", (P, W), F32, kind="ExternalInput").ap()
    w1t = nc.dram_tensor("w1t", (P, P), mdt, kind="ExternalInput").ap()
    w2t = nc.dram_tensor("w2t", (P, P), mdt, kind="ExternalInput").ap()
    w3t = nc.dram_tensor("w3t", (P, 1), mdt, kind="ExternalInput").ap()
    b1 = nc.dram_tensor("b1c", (P, 1), F32, kind="ExternalInput").ap()
    b2 = nc.dram_tensor("b2c", (P, 1), F32, kind="ExternalInput").ap()
    scal = nc.dram_tensor("scal", (n_groups, NG), F32, kind="ExternalOutput").ap()
    S_out = nc.dram_tensor("S_out", (n_groups, W, ML2), F32, kind="ExternalOutput").ap()

    # [grp] -> [128, T, 640] view with node-tile t at free offset t*640
    embL2_r = embL2.rearrange("(g t p) m -> g p t m", g=n_groups, t=T, p=P)

    silu = mybir.ActivationFunctionType.Silu

    with tile.TileContext(nc) as tc:
        with (
            tc.tile_pool(name="const", bufs=1) as cpool,
            tc.tile_pool(name="x0p", bufs=3) as x0p,
            tc.tile_pool(name="el2p", bufs=3) as el2p,
            tc.tile_pool(name="hp", bufs=2) as hp,
            tc.tile_pool(name="ap", bufs=3) as apool,
            tc.tile_pool(name="stp", bufs=2) as stp,
            tc.tile_pool(name="ph1", bufs=2, space="PSUM") as ph1p,
            tc.tile_pool(name="ph2", bufs=2, space="PSUM") as ph2p,
            tc.tile_pool(name="psc", bufs=2, space="PSUM") as pscp,
            tc.tile_pool(name="pS", bufs=1, space="PSUM") as pSp,
        ):
            w1s = cpool.tile([P, P], mdt)
            w2s = cpool.tile([P, P], mdt)
            w3s = cpool.tile([P, 1], mdt)
            b1s = cpool.tile([P, 1], F32)
            b2s = cpool.tile([P, 1], F32)
            iotas = cpool.tile([P, W], F32)
            lgids = cpool.tile([P, n_groups * T], F32)
            nc.sync.dma_start(out=w1s[:], in_=w1t)
            nc.sync.dma_start(out=w2s[:], in_=w2t)
            nc.sync.dma_start(out=w3s[:], in_=w3t)
            nc.sync.dma_start(out=b1s[:], in_=b1)
            nc.sync.dma_start(out=b2s[:], in_=b2)
            nc.sync.dma_start(out=iotas[:], in_=iota_in)
            nc.sync.dma_start(out=lgids[:], in_=lgid)

            for grp in range(n_groups):
                x0c = x0p.tile([P, NG], mdt, tag="x0c")
                nc.sync.dma_start(out=x0c[:], in_=x0T[:, grp * NG:(grp + 1) * NG])
                el2c = el2p.tile([P, T * ML2], mdt, tag="el2c")
                nc.sync.dma_start(
                    out=el2c[:].rearrange("p (t m) -> p t m", t=T, m=ML2),
                    in_=embL2_r[grp])

                # ---- scalar (MLP) branch, 512 nodes at a time ----
                scst = stp.tile([1, NG], F32, tag="scst")
                for s in range(SUP):
                    nsl = slice(s * 512, (s + 1) * 512)
                    h1p = ph1p.tile([P, 512], F32, tag="h1p")
                    nc.tensor.matmul(h1p[:], w1s[:], x0c[:, nsl],
                                     start=True, stop=True)
                    h1s = hp.tile([P, 512], mdt, tag="h1s")
                    nc.scalar.activation(h1s[:], h1p[:], silu, bias=b1s[:])
                    h2p = ph2p.tile([P, 512], F32, tag="h2p")
                    nc.tensor.matmul(h2p[:], w2s[:], h1s[:],
                                     start=True, stop=True)
                    h2s = hp.tile([P, 512], mdt, tag="h2s")
                    nc.scalar.activation(h2s[:], h2p[:], silu, bias=b2s[:])
                    scp = pscp.tile([1, 512], F32, tag="scp")
                    nc.tensor.matmul(scp[:], w3s[:], h2s[:],
                                     start=True, stop=True)
                    nc.scalar.copy(out=scst[:, nsl], in_=scp[:])
                nc.sync.dma_start(out=scal[grp:grp + 1, :], in_=scst[:])

                # ---- l=2 branch with fused segment sum ----
                pSa = pSp.tile([W, 320], F32, tag="pSa")
                pSb = pSp.tile([W, 320], F32, tag="pSb")
                for t in range(T):
                    At = apool.tile([P, W], mdt, tag="At")
                    col = grp * T + t
                    nc.vector.tensor_scalar(
                        At[:], iotas[:], lgids[:, col:col + 1], None,
                        op0=mybir.AluOpType.is_equal)
                    r0 = el2c[:, t * ML2: t * ML2 + 320]
                    r1 = el2c[:, t * ML2 + 320: (t + 1) * ML2]
                    nc.tensor.matmul(pSa[:], At[:], r0,
                                     start=(t == 0), stop=(t == T - 1))
                    nc.tensor.matmul(pSb[:], At[:], r1,
                                     start=(t == 0), stop=(t == T - 1))
                Sst = stp.tile([W, ML2], F32, tag="Sst")
                nc.vector.tensor_copy(out=Sst[:, 0:320], in_=pSa[:])
                nc.vector.tensor_copy(out=Sst[:, 320:640], in_=pSb[:])
                nc.sync.dma_start(out=S_out[grp], in_=Sst[:])

    nc.compile()
    _BUILD_CACHE[key] = nc
    return nc


def _next_pow2(x):
    p = 8
    while p < x:
        p *= 2
    return p


def _host_reference(node_embedding, W1, b1, W2, b2, W3, b3, w_l2, batch, natoms):
    """Pure-numpy fallback (only used for pathological graph layouts)."""
    G = natoms.shape[0]
    inv = 1.0 / natoms.astype(np.float32)
    x = node_embedding[:, 0, :]
    h = x @ W1.T + b1
    h = h / (1.0 + np.exp(-h))
    h = h @ W2.T + b2
    h = h / (1.0 + np.exp(-h))
    ns = (h @ W3.T + b3)[:, 0]
    iso = np.bincount(batch, weights=ns, minlength=G).astype(np.float32) * inv
    nl2 = np.einsum("nmc,c->nm", node_embedding[:, 4:9, :], w_l2[0])
    aniso = np.zeros((G, 5), np.float32)
    np.add.at(aniso, batch, nl2)
    aniso *= inv[:, None]
    dec = np.concatenate([iso[:, None], np.zeros((G, 3), np.float32), aniso], axis=1)
    return (dec @ _CG).reshape(-1, 3, 3).astype(np.float32)


def kernel(node_embedding, W1, b1, W2, b2, W3, b3, w_l2, batch, natoms):
    node_embedding = np.asarray(node_embedding, dtype=np.float32)
    W1 = np.asarray(W1, dtype=np.float32)
    b1 = np.asarray(b1, dtype=np.float32)
    W2 = np.asarray(W2, dtype=np.float32)
    b2 = np.asarray(b2, dtype=np.float32)
    W3 = np.asarray(W3, dtype=np.float32)
    b3 = np.asarray(b3, dtype=np.float32)
    w_l2 = np.asarray(w_l2, dtype=np.float32)
    batch = np.asarray(batch).astype(np.int64)
    natoms_in = np.asarray(natoms)

    N = node_embedding.shape[0]
    G = natoms_in.shape[0]
    n_sh = (N + N_CORES - 1) // N_CORES
    n_groups = (n_sh + NG - 1) // NG
    n_pad = n_groups * NG

    # per-core shard ranges and group graph bases
    shards = []
    W_need = 8
    ok = True
    for c in range(N_CORES):
        n0 = min(c * n_sh, N)
        n1 = min(n0 + n_sh, N)
        b = batch[n0:n1]
        nreal = n1 - n0
        gbase = np.zeros(n_groups, np.int64)
        for grp in range(n_groups):
            lo = grp * NG
            hi = min(lo + NG, nreal)
            if lo < nreal:
                gbase[grp] = b[lo]
                span = int(b[hi - 1] - b[lo] + 1)
                W_need = max(W_need, span)
        shards.append((n0, n1, b, gbase))
    W = _next_pow2(W_need)
    if W > P or not np.all(batch[:-1] <= batch[1:]):
        return _host_reference(node_embedding, W1, b1, W2, b2, W3, b3,
                               w_l2, batch, natoms_in)

    nc = _build(n_pad, n_groups, W)

    w1t = np.ascontiguousarray(W1.T)
    w2t = np.ascontiguousarray(W2.T)
    w3t = np.ascontiguousarray(W3.T)
    b1c = np.ascontiguousarray(b1[:, None])
    b2c = np.ascontiguousarray(b2[:, None])
    iota_c = np.tile(np.arange(W, dtype=np.float32), (P, 1))
    iota_c = np.ascontiguousarray(iota_c)

    in_maps = []
    for c in range(N_CORES):
        n0, n1, b, gbase = shards[c]
        nreal = n1 - n0
        x0T = np.zeros((P, n_pad), np.float32)
        x0T[:, :nreal] = node_embedding[n0:n1, 0, :].T
        el2 = np.zeros((n_pad, ML2), np.float32)
        el2[:nreal] = node_embedding[n0:n1, 4:9, :].reshape(nreal, ML2)
        lg = np.zeros(n_pad, np.float32)
        lg[:nreal] = (b - np.repeat(gbase, NG)[:nreal]).astype(np.float32)
        lg_t = np.ascontiguousarray(
            lg.reshape(n_groups, T, P).transpose(2, 0, 1).reshape(P, n_groups * T))
        in_maps.append({
            "x0T": x0T, "embL2": el2, "lgid": lg_t, "iota_in": iota_c,
            "w1t": w1t, "w2t": w2t, "w3t": w3t, "b1c": b1c, "b2c": b2c,
        })

    res = bass_utils.run_bass_kernel_spmd(nc, in_maps, core_ids=list(range(N_CORES)))

    # ---- host epilogue ----
    inv = (1.0 / natoms_in.astype(np.float32)).astype(np.float32)
    node_scalar = np.empty(N, np.float32)
    Sfull = np.zeros((G + P, ML2), np.float32)
    for c in range(N_CORES):
        n0, n1, _, gbase = shards[c]
        nreal = n1 - n0
        sc = res.results[c]["scal"].reshape(-1)[:nreal]
        node_scalar[n0:n1] = sc
        Sc = res.results[c]["S_out"]
        for grp in range(n_groups):
            if grp * NG < nreal:
                gb = int(gbase[grp])
                Sfull[gb:gb + W] += Sc[grp]
    iso = np.bincount(batch, weights=node_scalar + b3[0], minlength=G)
    iso = iso.astype(np.float32) * inv
    aniso = (Sfull[:G].reshape(G, 5, P) @ w_l2[0]) * inv[:, None]
    dec = np.concatenate([iso[:, None], np.zeros((G, 3), np.float32), aniso], axis=1)
    return (dec @ _CG).reshape(-1, 3, 3).astype(np.float32)


# revision 5
# speedup vs baseline: 1.0024x; 1.0024x over previous
"""CartBasisStressHead kernel for Trainium2 (8 NeuronCores, SPMD data-parallel).

Strategy
--------
Only 6 of the 9 m-rows of node_embedding are used: row 0 feeds a SiLU MLP
(per-node scalar), rows 4:9 feed a per-channel contraction (l=2 branch).
Nodes are sharded contiguously across 8 cores (graphs may straddle shard
boundaries; segment sums are linear, so host adds the partials).

Per core the device computes, in 1024-node groups:
  * scalar branch: h = silu(W1 @ x0T + b1); h = silu(W2 @ h + b2);
    s = W3 @ h  -> per-node scalars (feature-major layout, x0 transposed
    on host so the contraction dim lands on SBUF partitions).
  * l=2 branch fused with the segment sum: a 0/1 indicator matrix
    A[node, local_graph] (built on the vector engine from iota == local
    graph id) is the stationary matmul operand, so
    S[g_local, (m,c)] = sum_n A[n,g] * emb[n, (m,c)] accumulates in PSUM
    across the group's 8 node-tiles. Data streams through the PE once.

Host epilogue: bincount segment-sum of per-node scalars, scatter-add of
per-group S partials, contraction with w_l2, and the tiny (G,9)@(9,9)
change-of-basis.
"""

import sys

if "/opt/trn_rl_repo" not in sys.path:
    sys.path.insert(0, "/opt/trn_rl_repo")

import numpy as np

import concourse.bacc as bacc
import concourse.tile as tile
from concourse import mybir
from concourse import bass_utils

_S2 = 2.0 ** -0.5
_S3 = 3.0 ** -0.5
_S6 = 6.0 ** -0.5
_CG = np.array([
    [_S3, 0, 0, 0, _S3, 0, 0, 0, _S3],
    [0, 0, 0, 0, 0, _S2, 0, -_S2, 0],
    [0, 0, -_S2, 0, 0, 0, _S2, 0, 0],
    [0, _S2, 0, -_S2, 0, 0, 0, 0, 0],
    [0, 0, _S2, 0, 0, 0, _S2, 0, 0],
    [0, 0, 0, 0, 0, _S2, 0, _S2, 0],
    [-_S6, 0, 0, 0, 2 * _S6, 0, 0, 0, -_S6],
    [0, _S2, 0, _S2, 0, 0, 0, 0, 0],
    [-_S2, 0, 0, 0, 0, 0, 0, 0, _S2],
], dtype=np.float32)  # (9, 9)

N_CORES = 8
P = 128          # SBUF partitions
NG = 1024        # nodes per group (one PSUM accumulation span)
T = NG // P      # node-tiles per group
SUP = NG // 512  # 512-node supertiles per group (MLP granularity)
ML2 = 5 * P      # 640 floats of l=2 data per node

F32 = mybir.dt.float32
F32R = mybir.dt.float32r

_BUILD_CACHE = {}


def _build(n_pad, n_groups, W, n_real, use_f32r=True):
    key = (n_pad, n_groups, W, n_real, use_f32r)
    if key in _BUILD_CACHE:
        return _BUILD_CACHE[key]

    nc = bacc.Bacc("TRN2", target_bir_lowering=False, debug=False,
                   num_devices=N_CORES)

    mdt = F32R if use_f32r else F32
    x0T = nc.dram_tensor("x0T", (P, n_pad), mdt, kind="ExternalInput").ap()
    embL2 = nc.dram_tensor("embL2", (n_pad, ML2), mdt, kind="ExternalInput").ap()
    lgid = nc.dram_tensor("lgid", (P, n_groups * T), F32, kind="ExternalInput").ap()
    iota_in = nc.dram_tensor("iota_in", (P, W), F32, kind="ExternalInput").ap()
    w1t = nc.dram_tensor("w1t", (P, P), mdt, kind="ExternalInput").ap()
    w2t = nc.dram_tensor("w2t", (P, P), mdt, kind="ExternalInput").ap()
    w3t = nc.dram_tensor("w3t", (P, 1), mdt, kind="ExternalInput").ap()
    b1 = nc.dram_tensor("b1c", (P, 1), F32, kind="ExternalInput").ap()
    b2 = nc.dram_tensor("b2c", (P, 1), F32, kind="ExternalInput").ap()
    scal = nc.dram_tensor("scal", (n_groups, NG), F32, kind="ExternalOutput").ap()
    S_out = nc.dram_tensor("S_out", (n_groups, W, ML2), F32, kind="ExternalOutput").ap()

    # [grp] -> [128, T, 640] view with node-tile t at free offset t*640
    embL2_r = embL2.rearrange("(g t p) m -> g p t m", g=n_groups, t=T, p=P)

    silu = mybir.ActivationFunctionType.Silu

    with tile.TileContext(nc) as tc:
        with (
            tc.tile_pool(name="const", bufs=1) as cpool,
            tc.tile_pool(name="x0p", bufs=3) as x0p,
            tc.tile_pool(name="el2p", bufs=3) as el2p,
            tc.tile_pool(name="hp", bufs=2) as hp,
            tc.tile_pool(name="ap", bufs=3) as apool,
            tc.tile_pool(name="stp", bufs=2) as stp,
            tc.tile_pool(name="ph1", bufs=2, space="PSUM") as ph1p,
            tc.tile_pool(name="ph2", bufs=2, space="PSUM") as ph2p,
            tc.tile_pool(name="psc", bufs=2, space="PSUM") as pscp,
            tc.tile_pool(name="pS", bufs=1, space="PSUM") as pSp,
        ):
            w1s = cpool.tile([P, P], mdt)
            w2s = cpool.tile([P, P], mdt)
            w3s = cpool.tile([P, 1], mdt)
            b1s = cpool.tile([P, 1], F32)
            b2s = cpool.tile([P, 1], F32)
            iotas = cpool.tile([P, W], F32)
            lgids = cpool.tile([P, n_groups * T], F32)
            nc.sync.dma_start(out=w1s[:], in_=w1t)
            nc.sync.dma_start(out=w2s[:], in_=w2t)
            nc.sync.dma_start(out=w3s[:], in_=w3t)
            nc.sync.dma_start(out=b1s[:], in_=b1)
            nc.sync.dma_start(out=b2s[:], in_=b2)
            nc.sync.dma_start(out=iotas[:], in_=iota_in)
            nc.sync.dma_start(out=lgids[:], in_=lgid)

            for grp in range(n_groups):
                # tail group: only load/compute tiles that contain real nodes
                grp_real = min(NG, n_real - grp * NG)
                Tr = (grp_real + P - 1) // P
                Sr = (grp_real + 511) // 512
                x0c = x0p.tile([P, NG], mdt, tag="x0c")
                nc.sync.dma_start(out=x0c[:, :Sr * 512],
                                  in_=x0T[:, grp * NG: grp * NG + Sr * 512])
                el2c = el2p.tile([P, T * ML2], mdt, tag="el2c")
                nc.sync.dma_start(
                    out=el2c[:, :Tr * ML2].rearrange("p (t m) -> p t m", t=Tr,
                                                     m=ML2),
                    in_=embL2_r[grp][:, 0:Tr, :])

                # ---- scalar (MLP) branch, 512 nodes at a time ----
                scst = stp.tile([1, NG], F32, tag="scst")
                for s in range(Sr):
                    nsl = slice(s * 512, (s + 1) * 512)
                    h1p = ph1p.tile([P, 512], F32, tag="h1p")
                    nc.tensor.matmul(h1p[:], w1s[:], x0c[:, nsl],
                                     start=True, stop=True)
                    h1s = hp.tile([P, 512], mdt, tag="h1s")
                    nc.scalar.activation(h1s[:], h1p[:], silu, bias=b1s[:])
                    h2p = ph2p.tile([P, 512], F32, tag="h2p")
                    nc.tensor.matmul(h2p[:], w2s[:], h1s[:],
                                     start=True, stop=True)
                    h2s = hp.tile([P, 512], mdt, tag="h2s")
                    nc.scalar.activation(h2s[:], h2p[:], silu, bias=b2s[:])
                    scp = pscp.tile([1, 512], F32, tag="scp")
                    nc.tensor.matmul(scp[:], w3s[:], h2s[:],
                                     start=True, stop=True)
                    nc.scalar.copy(out=scst[:, nsl], in_=scp[:])
                nc.gpsimd.dma_start(out=scal[grp:grp + 1, :Sr * 512],
                                    in_=scst[:, :Sr * 512])

                # ---- l=2 branch with fused segment sum ----
                pSa = pSp.tile([W, 320], F32, tag="pSa")
                pSb = pSp.tile([W, 320], F32, tag="pSb")
                for t in range(Tr):
                    At = apool.tile([P, W], mdt, tag="At")
                    col = grp * T + t
                    nc.vector.tensor_scalar(
                        At[:], iotas[:], lgids[:, col:col + 1], None,
                        op0=mybir.AluOpType.is_equal)
                    r0 = el2c[:, t * ML2: t * ML2 + 320]
                    r1 = el2c[:, t * ML2 + 320: (t + 1) * ML2]
                    nc.tensor.matmul(pSa[:], At[:], r0,
                                     start=(t == 0), stop=(t == Tr - 1))
                    nc.tensor.matmul(pSb[:], At[:], r1,
                                     start=(t == 0), stop=(t == Tr - 1))
                Sst = stp.tile([W, ML2], F32, tag="Sst")
                nc.vector.tensor_copy(out=Sst[:, 0:320], in_=pSa[:])
                nc.vector.tensor_copy(out=Sst[:, 320:640], in_=pSb[:])
                nc.gpsimd.dma_start(out=S_out[grp], in_=Sst[:])

    nc.compile()
    _BUILD_CACHE[key] = nc
    return nc


def _next_pow2(x):
    p = 8
    while p < x:
        p *= 2
    return p


def _host_reference(node_embedding, W1, b1, W2, b2, W3, b3, w_l2, batch, natoms):
    """Pure-numpy fallback (only used for pathological graph layouts)."""
    G = natoms.shape[0]
    inv = 1.0 / natoms.astype(np.float32)
    x = node_embedding[:, 0, :]
    h = x @ W1.T + b1
    h = h / (1.0 + np.exp(-h))
    h = h @ W2.T + b2
    h = h / (1.0 + np.exp(-h))
    ns = (h @ W3.T + b3)[:, 0]
    iso = np.bincount(batch, weights=ns, minlength=G).astype(np.float32) * inv
    nl2 = np.einsum("nmc,c->nm", node_embedding[:, 4:9, :], w_l2[0])
    aniso = np.zeros((G, 5), np.float32)
    np.add.at(aniso, batch, nl2)
    aniso *= inv[:, None]
    dec = np.concatenate([iso[:, None], np.zeros((G, 3), np.float32), aniso], axis=1)
    return (dec @ _CG).reshape(-1, 3, 3).astype(np.float32)


def kernel(node_embedding, W1, b1, W2, b2, W3, b3, w_l2, batch, natoms):
    node_embedding = np.asarray(node_embedding, dtype=np.float32)
    W1 = np.asarray(W1, dtype=np.float32)
    b1 = np.asarray(b1, dtype=np.float32)
    W2 = np.asarray(W2, dtype=np.float32)
    b2 = np.asarray(b2, dtype=np.float32)
    W3 = np.asarray(W3, dtype=np.float32)
    b3 = np.asarray(b3, dtype=np.float32)
    w_l2 = np.asarray(w_l2, dtype=np.float32)
    batch = np.asarray(batch).astype(np.int64)
    natoms_in = np.asarray(natoms)

    N = node_embedding.shape[0]
    G = natoms_in.shape[0]
    n_sh = (N + N_CORES - 1) // N_CORES
    n_groups = (n_sh + NG - 1) // NG
    n_pad = n_groups * NG

    # per-core shard ranges and group graph bases
    shards = []
    W_need = 8
    ok = True
    for c in range(N_CORES):
        n0 = min(c * n_sh, N)
        n1 = min(n0 + n_sh, N)
        b = batch[n0:n1]
        nreal = n1 - n0
        gbase = np.zeros(n_groups, np.int64)
        for grp in range(n_groups):
            lo = grp * NG
            hi = min(lo + NG, nreal)
            if lo < nreal:
                gbase[grp] = b[lo]
                span = int(b[hi - 1] - b[lo] + 1)
                W_need = max(W_need, span)
        shards.append((n0, n1, b, gbase))
    W = _next_pow2(W_need)
    if W > P or not np.all(batch[:-1] <= batch[1:]):
        return _host_reference(node_embedding, W1, b1, W2, b2, W3, b3,
                               w_l2, batch, natoms_in)

    nc = _build(n_pad, n_groups, W, n_sh)

    w1t = np.ascontiguousarray(W1.T)
    w2t = np.ascontiguousarray(W2.T)
    w3t = np.ascontiguousarray(W3.T)
    b1c = np.ascontiguousarray(b1[:, None])
    b2c = np.ascontiguousarray(b2[:, None])
    iota_c = np.tile(np.arange(W, dtype=np.float32), (P, 1))
    iota_c = np.ascontiguousarray(iota_c)

    in_maps = []
    for c in range(N_CORES):
        n0, n1, b, gbase = shards[c]
        nreal = n1 - n0
        x0T = np.zeros((P, n_pad), np.float32)
        x0T[:, :nreal] = node_embedding[n0:n1, 0, :].T
        el2 = np.zeros((n_pad, ML2), np.float32)
        el2[:nreal] = node_embedding[n0:n1, 4:9, :].reshape(nreal, ML2)
        lg = np.full(n_pad, -1.0, np.float32)
        lg[:nreal] = (b - np.repeat(gbase, NG)[:nreal]).astype(np.float32)
        lg_t = np.ascontiguousarray(
            lg.reshape(n_groups, T, P).transpose(2, 0, 1).reshape(P, n_groups * T))
        in_maps.append({
            "x0T": x0T, "embL2": el2, "lgid": lg_t, "iota_in": iota_c,
            "w1t": w1t, "w2t": w2t, "w3t": w3t, "b1c": b1c, "b2c": b2c,
        })

    res = bass_utils.run_bass_kernel_spmd(nc, in_maps, core_ids=list(range(N_CORES)))

    # ---- host epilogue ----
    inv = (1.0 / natoms_in.astype(np.float32)).astype(np.float32)
    node_scalar = np.empty(N, np.float32)
    Sfull = np.zeros((G + P, ML2), np.float32)
    for c in range(N_CORES):
        n0, n1, _, gbase = shards[c]
        nreal = n1 - n0
        sc = res.results[c]["scal"].reshape(-1)[:nreal]
        node_scalar[n0:n1] = sc
        Sc = res.results[c]["S_out"]
        for grp in range(n_groups):
            if grp * NG < nreal:
                gb = int(gbase[grp])
                Sfull[gb:gb + W] += Sc[grp]
    iso = np.bincount(batch, weights=node_scalar + b3[0], minlength=G)
    iso = iso.astype(np.float32) * inv
    aniso = (Sfull[:G].reshape(G, 5, P) @ w_l2[0]) * inv[:, None]
    dec = np.concatenate([iso[:, None], np.zeros((G, 3), np.float32), aniso], axis=1)
    return (dec @ _CG).reshape(-1, 3, 3).astype(np.float32)


# revision 6
# speedup vs baseline: 1.7745x; 1.7702x over previous
"""CartBasisStressHead kernel for Trainium2 (8 NeuronCores, SPMD data-parallel).

Strategy
--------
Only 6 of the 9 m-rows of node_embedding are used: row 0 feeds a SiLU MLP
(per-node scalar), rows 4:9 feed a per-channel contraction (l=2 branch).
Nodes are sharded contiguously across 8 cores (graphs may straddle shard
boundaries; segment sums are linear, so host adds the partials).

Per core the device computes, in 1024-node groups:
  * scalar branch: h = silu(W1 @ x0T + b1); h = silu(W2 @ h + b2);
    s = W3 @ h  -> per-node scalars (feature-major layout, x0 transposed
    on host so the contraction dim lands on SBUF partitions).
  * l=2 branch fused with the segment sum: a 0/1 indicator matrix
    A[node, local_graph] (built on the vector engine from iota == local
    graph id) is the stationary matmul operand, so
    S[g_local, (m,c)] = sum_n A[n,g] * emb[n, (m,c)] accumulates in PSUM
    across the group's 8 node-tiles. Data streams through the PE once.

Host epilogue: bincount segment-sum of per-node scalars, scatter-add of
per-group S partials, contraction with w_l2, and the tiny (G,9)@(9,9)
change-of-basis.
"""

import sys

if "/opt/trn_rl_repo" not in sys.path:
    sys.path.insert(0, "/opt/trn_rl_repo")

import numpy as np
import ml_dtypes

import concourse.bacc as bacc
import concourse.tile as tile
from concourse import mybir
from concourse import bass_utils

_S2 = 2.0 ** -0.5
_S3 = 3.0 ** -0.5
_S6 = 6.0 ** -0.5
_CG = np.array([
    [_S3, 0, 0, 0, _S3, 0, 0, 0, _S3],
    [0, 0, 0, 0, 0, _S2, 0, -_S2, 0],
    [0, 0, -_S2, 0, 0, 0, _S2, 0, 0],
    [0, _S2, 0, -_S2, 0, 0, 0, 0, 0],
    [0, 0, _S2, 0, 0, 0, _S2, 0, 0],
    [0, 0, 0, 0, 0, _S2, 0, _S2, 0],
    [-_S6, 0, 0, 0, 2 * _S6, 0, 0, 0, -_S6],
    [0, _S2, 0, _S2, 0, 0, 0, 0, 0],
    [-_S2, 0, 0, 0, 0, 0, 0, 0, _S2],
], dtype=np.float32)  # (9, 9)

N_CORES = 8
P = 128          # SBUF partitions
NG = 1024        # nodes per group (one PSUM accumulation span)
T = NG // P      # node-tiles per group
SUP = NG // 512  # 512-node supertiles per group (MLP granularity)
ML2 = 5 * P      # 640 floats of l=2 data per node

F32 = mybir.dt.float32
F32R = mybir.dt.float32r
BF16 = mybir.dt.bfloat16

_BUILD_CACHE = {}


def _build(n_pad, n_groups, W, n_real, mm_dt="bf16"):
    key = (n_pad, n_groups, W, n_real, mm_dt)
    if key in _BUILD_CACHE:
        return _BUILD_CACHE[key]

    nc = bacc.Bacc("TRN2", target_bir_lowering=False, debug=False,
                   num_devices=N_CORES)

    mdt = {"bf16": BF16, "f32r": F32R, "f32": F32}[mm_dt]
    x0T = nc.dram_tensor("x0T", (P, n_pad), mdt, kind="ExternalInput").ap()
    embL2 = nc.dram_tensor("embL2", (n_pad, ML2), mdt, kind="ExternalInput").ap()
    lgid = nc.dram_tensor("lgid", (P, n_groups * T), F32, kind="ExternalInput").ap()
    iota_in = nc.dram_tensor("iota_in", (P, W), F32, kind="ExternalInput").ap()
    w1t = nc.dram_tensor("w1t", (P, P), mdt, kind="ExternalInput").ap()
    w2t = nc.dram_tensor("w2t", (P, P), mdt, kind="ExternalInput").ap()
    w3t = nc.dram_tensor("w3t", (P, 1), mdt, kind="ExternalInput").ap()
    b1 = nc.dram_tensor("b1c", (P, 1), F32, kind="ExternalInput").ap()
    b2 = nc.dram_tensor("b2c", (P, 1), F32, kind="ExternalInput").ap()
    scal = nc.dram_tensor("scal", (n_groups, NG), F32, kind="ExternalOutput").ap()
    S_out = nc.dram_tensor("S_out", (n_groups, W, ML2), F32, kind="ExternalOutput").ap()

    # [grp] -> [128, T, 640] view with node-tile t at free offset t*640
    embL2_r = embL2.rearrange("(g t p) m -> g p t m", g=n_groups, t=T, p=P)

    silu = mybir.ActivationFunctionType.Silu

    with tile.TileContext(nc) as tc:
        with (
            tc.tile_pool(name="const", bufs=1) as cpool,
            tc.tile_pool(name="x0p", bufs=3) as x0p,
            tc.tile_pool(name="el2p", bufs=3) as el2p,
            tc.tile_pool(name="hp", bufs=2) as hp,
            tc.tile_pool(name="ap", bufs=3) as apool,
            tc.tile_pool(name="stp", bufs=2) as stp,
            tc.tile_pool(name="ph1", bufs=2, space="PSUM") as ph1p,
            tc.tile_pool(name="ph2", bufs=2, space="PSUM") as ph2p,
            tc.tile_pool(name="psc", bufs=2, space="PSUM") as pscp,
            tc.tile_pool(name="pS", bufs=1, space="PSUM") as pSp,
        ):
            w1s = cpool.tile([P, P], mdt)
            w2s = cpool.tile([P, P], mdt)
            w3s = cpool.tile([P, 1], mdt)
            b1s = cpool.tile([P, 1], F32)
            b2s = cpool.tile([P, 1], F32)
            iotas = cpool.tile([P, W], F32)
            lgids = cpool.tile([P, n_groups * T], F32)
            nc.sync.dma_start(out=w1s[:], in_=w1t)
            nc.sync.dma_start(out=w2s[:], in_=w2t)
            nc.sync.dma_start(out=w3s[:], in_=w3t)
            nc.sync.dma_start(out=b1s[:], in_=b1)
            nc.sync.dma_start(out=b2s[:], in_=b2)
            nc.sync.dma_start(out=iotas[:], in_=iota_in)
            nc.sync.dma_start(out=lgids[:], in_=lgid)

            for grp in range(n_groups):
                # tail group: only load/compute tiles that contain real nodes
                grp_real = min(NG, n_real - grp * NG)
                Tr = (grp_real + P - 1) // P
                Sr = (grp_real + 511) // 512
                x0c = x0p.tile([P, NG], mdt, tag="x0c")
                nc.sync.dma_start(out=x0c[:, :Sr * 512],
                                  in_=x0T[:, grp * NG: grp * NG + Sr * 512])
                el2c = el2p.tile([P, T * ML2], mdt, tag="el2c")
                nc.sync.dma_start(
                    out=el2c[:, :Tr * ML2].rearrange("p (t m) -> p t m", t=Tr,
                                                     m=ML2),
                    in_=embL2_r[grp][:, 0:Tr, :])

                # ---- scalar (MLP) branch, 512 nodes at a time ----
                scst = stp.tile([1, NG], F32, tag="scst")
                for s in range(Sr):
                    nsl = slice(s * 512, (s + 1) * 512)
                    h1p = ph1p.tile([P, 512], F32, tag="h1p")
                    nc.tensor.matmul(h1p[:], w1s[:], x0c[:, nsl],
                                     start=True, stop=True)
                    h1s = hp.tile([P, 512], mdt, tag="h1s")
                    nc.scalar.activation(h1s[:], h1p[:], silu, bias=b1s[:])
                    h2p = ph2p.tile([P, 512], F32, tag="h2p")
                    nc.tensor.matmul(h2p[:], w2s[:], h1s[:],
                                     start=True, stop=True)
                    h2s = hp.tile([P, 512], mdt, tag="h2s")
                    nc.scalar.activation(h2s[:], h2p[:], silu, bias=b2s[:])
                    scp = pscp.tile([1, 512], F32, tag="scp")
                    nc.tensor.matmul(scp[:], w3s[:], h2s[:],
                                     start=True, stop=True)
                    nc.scalar.copy(out=scst[:, nsl], in_=scp[:])
                nc.gpsimd.dma_start(out=scal[grp:grp + 1, :Sr * 512],
                                    in_=scst[:, :Sr * 512])

                # ---- l=2 branch with fused segment sum ----
                pSa = pSp.tile([W, 320], F32, tag="pSa")
                pSb = pSp.tile([W, 320], F32, tag="pSb")
                for t in range(Tr):
                    At = apool.tile([P, W], mdt, tag="At")
                    col = grp * T + t
                    nc.vector.tensor_scalar(
                        At[:], iotas[:], lgids[:, col:col + 1], None,
                        op0=mybir.AluOpType.is_equal)
                    r0 = el2c[:, t * ML2: t * ML2 + 320]
                    r1 = el2c[:, t * ML2 + 320: (t + 1) * ML2]
                    nc.tensor.matmul(pSa[:], At[:], r0,
                                     start=(t == 0), stop=(t == Tr - 1))
                    nc.tensor.matmul(pSb[:], At[:], r1,
                                     start=(t == 0), stop=(t == Tr - 1))
                Sst = stp.tile([W, ML2], F32, tag="Sst")
                nc.vector.tensor_copy(out=Sst[:, 0:320], in_=pSa[:])
                nc.vector.tensor_copy(out=Sst[:, 320:640], in_=pSb[:])
                nc.gpsimd.dma_start(out=S_out[grp], in_=Sst[:])

    nc.compile()
    _BUILD_CACHE[key] = nc
    return nc


def _next_pow2(x):
    p = 8
    while p < x:
        p *= 2
    return p


def _host_reference(node_embedding, W1, b1, W2, b2, W3, b3, w_l2, batch, natoms):
    """Pure-numpy fallback (only used for pathological graph layouts)."""
    G = natoms.shape[0]
    inv = 1.0 / natoms.astype(np.float32)
    x = node_embedding[:, 0, :]
    h = x @ W1.T + b1
    h = h / (1.0 + np.exp(-h))
    h = h @ W2.T + b2
    h = h / (1.0 + np.exp(-h))
    ns = (h @ W3.T + b3)[:, 0]
    iso = np.bincount(batch, weights=ns, minlength=G).astype(np.float32) * inv
    nl2 = np.einsum("nmc,c->nm", node_embedding[:, 4:9, :], w_l2[0])
    aniso = np.zeros((G, 5), np.float32)
    np.add.at(aniso, batch, nl2)
    aniso *= inv[:, None]
    dec = np.concatenate([iso[:, None], np.zeros((G, 3), np.float32), aniso], axis=1)
    return (dec @ _CG).reshape(-1, 3, 3).astype(np.float32)


def kernel(node_embedding, W1, b1, W2, b2, W3, b3, w_l2, batch, natoms):
    node_embedding = np.asarray(node_embedding, dtype=np.float32)
    W1 = np.asarray(W1, dtype=np.float32)
    b1 = np.asarray(b1, dtype=np.float32)
    W2 = np.asarray(W2, dtype=np.float32)
    b2 = np.asarray(b2, dtype=np.float32)
    W3 = np.asarray(W3, dtype=np.float32)
    b3 = np.asarray(b3, dtype=np.float32)
    w_l2 = np.asarray(w_l2, dtype=np.float32)
    batch = np.asarray(batch).astype(np.int64)
    natoms_in = np.asarray(natoms)

    N = node_embedding.shape[0]
    G = natoms_in.shape[0]
    n_sh = (N + N_CORES - 1) // N_CORES
    n_groups = (n_sh + NG - 1) // NG
    n_pad = n_groups * NG

    # per-core shard ranges and group graph bases
    shards = []
    W_need = 8
    ok = True
    for c in range(N_CORES):
        n0 = min(c * n_sh, N)
        n1 = min(n0 + n_sh, N)
        b = batch[n0:n1]
        nreal = n1 - n0
        gbase = np.zeros(n_groups, np.int64)
        for grp in range(n_groups):
            lo = grp * NG
            hi = min(lo + NG, nreal)
            if lo < nreal:
                gbase[grp] = b[lo]
                span = int(b[hi - 1] - b[lo] + 1)
                W_need = max(W_need, span)
        shards.append((n0, n1, b, gbase))
    W = _next_pow2(W_need)
    if W > P or not np.all(batch[:-1] <= batch[1:]):
        return _host_reference(node_embedding, W1, b1, W2, b2, W3, b3,
                               w_l2, batch, natoms_in)

    mm_dt = "bf16"
    wire = {"bf16": ml_dtypes.bfloat16, "f32r": np.float32,
            "f32": np.float32}[mm_dt]
    nc = _build(n_pad, n_groups, W, n_sh, mm_dt)

    w1t = np.ascontiguousarray(W1.T).astype(wire)
    w2t = np.ascontiguousarray(W2.T).astype(wire)
    w3t = np.ascontiguousarray(W3.T).astype(wire)
    b1c = np.ascontiguousarray(b1[:, None])
    b2c = np.ascontiguousarray(b2[:, None])
    iota_c = np.tile(np.arange(W, dtype=np.float32), (P, 1))
    iota_c = np.ascontiguousarray(iota_c)

    in_maps = []
    for c in range(N_CORES):
        n0, n1, b, gbase = shards[c]
        nreal = n1 - n0
        x0T = np.zeros((P, n_pad), wire)
        x0T[:, :nreal] = node_embedding[n0:n1, 0, :].T.astype(wire)
        el2 = np.zeros((n_pad, ML2), wire)
        el2[:nreal] = node_embedding[n0:n1, 4:9, :].reshape(nreal, ML2).astype(wire)
        lg = np.full(n_pad, -1.0, np.float32)
        lg[:nreal] = (b - np.repeat(gbase, NG)[:nreal]).astype(np.float32)
        lg_t = np.ascontiguousarray(
            lg.reshape(n_groups, T, P).transpose(2, 0, 1).reshape(P, n_groups * T))
        in_maps.append({
            "x0T": x0T, "embL2": el2, "lgid": lg_t, "iota_in": iota_c,
            "w1t": w1t, "w2t": w2t, "w3t": w3t, "b1c": b1c, "b2c": b2c,
        })

    res = bass_utils.run_bass_kernel_spmd(nc, in_maps, core_ids=list(range(N_CORES)))

    # ---- host epilogue ----
    inv = (1.0 / natoms_in.astype(np.float32)).astype(np.float32)
    node_scalar = np.empty(N, np.float32)
    Sfull = np.zeros((G + P, ML2), np.float32)
    for c in range(N_CORES):
        n0, n1, _, gbase = shards[c]
        nreal = n1 - n0
        sc = res.results[c]["scal"].reshape(-1)[:nreal]
        node_scalar[n0:n1] = sc
        Sc = res.results[c]["S_out"]
        for grp in range(n_groups):
            if grp * NG < nreal:
                gb = int(gbase[grp])
                Sfull[gb:gb + W] += Sc[grp]
    iso = np.bincount(batch, weights=node_scalar + b3[0], minlength=G)
    iso = iso.astype(np.float32) * inv
    aniso = (Sfull[:G].reshape(G, 5, P) @ w_l2[0]) * inv[:, None]
    dec = np.concatenate([iso[:, None], np.zeros((G, 3), np.float32), aniso], axis=1)
    return (dec @ _CG).reshape(-1, 3, 3).astype(np.float32)


# revision 8
# speedup vs baseline: 1.8156x; 1.0232x over previous
"""CartBasisStressHead kernel for Trainium2 (8 NeuronCores, SPMD data-parallel).

Strategy
--------
Only 6 of the 9 m-rows of node_embedding are used: row 0 feeds a SiLU MLP
(per-node scalar), rows 4:9 feed a per-channel contraction (l=2 branch).
Nodes are sharded contiguously across 8 cores (graphs may straddle shard
boundaries; segment sums are linear, so host adds the partials).

Per core the device computes, in 1024-node groups:
  * scalar branch: h = silu(W1 @ x0T + b1); h = silu(W2 @ h + b2);
    s = W3 @ h  -> per-node scalars (feature-major layout, x0 transposed
    on host so the contraction dim lands on SBUF partitions).
  * l=2 branch fused with the segment sum: a 0/1 indicator matrix
    A[node, local_graph] (built on the vector engine from iota == local
    graph id) is the stationary matmul operand, so
    S[g_local, (m,c)] = sum_n A[n,g] * emb[n, (m,c)] accumulates in PSUM
    across the group's 8 node-tiles. Data streams through the PE once.

Matmul data travels in bf16 (fp32 accumulation in PSUM). Inputs are loaded
in 2-group (~2.6 MB) chunks, outputs staged in SBUF over 4 groups before a
single store, so nearly all DMA time is large transfers.

Host epilogue: bincount segment-sum of per-node scalars, scatter-add of
per-group S partials, contraction with w_l2, and the tiny (G,9)@(9,9)
change-of-basis.
"""

import sys

if "/opt/trn_rl_repo" not in sys.path:
    sys.path.insert(0, "/opt/trn_rl_repo")

import numpy as np
import ml_dtypes

import concourse.bacc as bacc
import concourse.tile as tile
from concourse import mybir
from concourse import bass_utils

_S2 = 2.0 ** -0.5
_S3 = 3.0 ** -0.5
_S6 = 6.0 ** -0.5
_CG = np.array([
    [_S3, 0, 0, 0, _S3, 0, 0, 0, _S3],
    [0, 0, 0, 0, 0, _S2, 0, -_S2, 0],
    [0, 0, -_S2, 0, 0, 0, _S2, 0, 0],
    [0, _S2, 0, -_S2, 0, 0, 0, 0, 0],
    [0, 0, _S2, 0, 0, 0, _S2, 0, 0],
    [0, 0, 0, 0, 0, _S2, 0, _S2, 0],
    [-_S6, 0, 0, 0, 2 * _S6, 0, 0, 0, -_S6],
    [0, _S2, 0, _S2, 0, 0, 0, 0, 0],
    [-_S2, 0, 0, 0, 0, 0, 0, 0, _S2],
], dtype=np.float32)  # (9, 9)

N_CORES = 8
P = 128          # SBUF partitions
NG = 1024        # nodes per group (one PSUM accumulation span)
T = NG // P      # node-tiles per group
ML2 = 5 * P      # 640 values of l=2 data per node
SC = 2           # groups per input superchunk
OB = 4           # groups per output staging batch

F32 = mybir.dt.float32
BF16 = mybir.dt.bfloat16
WIRE = ml_dtypes.bfloat16

_BUILD_CACHE = {}


def _build(n_pad, n_groups, W, n_real):
    key = (n_pad, n_groups, W, n_real)
    if key in _BUILD_CACHE:
        return _BUILD_CACHE[key]

    n_sc = (n_groups + SC - 1) // SC
    n_ob = (n_groups + OB - 1) // OB

    nc = bacc.Bacc("TRN2", target_bir_lowering=False, debug=False,
                   num_devices=N_CORES)

    x0T = nc.dram_tensor("x0T", (P, n_pad), BF16, kind="ExternalInput").ap()
    # host pre-tiled: [sc, p, sc_tile*640] fully contiguous per partition
    embL2 = nc.dram_tensor("embL2", (n_sc, P, SC * T * ML2), BF16,
                           kind="ExternalInput").ap()
    lgid = nc.dram_tensor("lgid", (P, n_groups * T), F32,
                          kind="ExternalInput").ap()
    iota_in = nc.dram_tensor("iota_in", (P, W), F32, kind="ExternalInput").ap()
    w1t = nc.dram_tensor("w1t", (P, P), BF16, kind="ExternalInput").ap()
    w2t = nc.dram_tensor("w2t", (P, P), BF16, kind="ExternalInput").ap()
    w3t = nc.dram_tensor("w3t", (P, 1), BF16, kind="ExternalInput").ap()
    b1 = nc.dram_tensor("b1c", (P, 1), F32, kind="ExternalInput").ap()
    b2 = nc.dram_tensor("b2c", (P, 1), F32, kind="ExternalInput").ap()
    scal = nc.dram_tensor("scal", (n_ob, OB * NG), F32,
                          kind="ExternalOutput").ap()
    S_out = nc.dram_tensor("S_out", (n_ob, W, OB * ML2), F32,
                           kind="ExternalOutput").ap()

    silu = mybir.ActivationFunctionType.Silu
    eq = mybir.AluOpType.is_equal

    with tile.TileContext(nc) as tc:
        with (
            tc.tile_pool(name="const", bufs=1) as cpool,
            tc.tile_pool(name="x0p", bufs=3) as x0p,
            tc.tile_pool(name="el2p", bufs=3) as el2p,
            tc.tile_pool(name="hp", bufs=2) as hp,
            tc.tile_pool(name="apl", bufs=2) as apool,
            tc.tile_pool(name="stp", bufs=2) as stp,
            tc.tile_pool(name="ph1", bufs=2, space="PSUM") as ph1p,
            tc.tile_pool(name="ph2", bufs=2, space="PSUM") as ph2p,
            tc.tile_pool(name="psc", bufs=2, space="PSUM") as pscp,
            tc.tile_pool(name="pS", bufs=1, space="PSUM") as pSp,
        ):
            w1s = cpool.tile([P, P], BF16)
            w2s = cpool.tile([P, P], BF16)
            w3s = cpool.tile([P, 1], BF16)
            b1s = cpool.tile([P, 1], F32)
            b2s = cpool.tile([P, 1], F32)
            iotas = cpool.tile([P, W], F32)
            lgids = cpool.tile([P, n_groups * T], F32)
            nc.sync.dma_start(out=w1s[:], in_=w1t)
            nc.sync.dma_start(out=w2s[:], in_=w2t)
            nc.sync.dma_start(out=w3s[:], in_=w3t)
            nc.sync.dma_start(out=b1s[:], in_=b1)
            nc.sync.dma_start(out=b2s[:], in_=b2)
            nc.sync.dma_start(out=iotas[:], in_=iota_in)
            nc.sync.dma_start(out=lgids[:], in_=lgid)

            x0c = el2c = None
            scst = Sst = None
            for grp in range(n_groups):
                grp_real = min(NG, n_real - grp * NG)
                Tr = (grp_real + P - 1) // P
                Sr = (grp_real + 511) // 512

                if grp % SC == 0:
                    # tiles with real nodes in this superchunk
                    sc_real = min(SC * NG, n_real - grp * NG)
                    Tsc = (sc_real + P - 1) // P
                    Ssc = (sc_real + 511) // 512
                    x0c = x0p.tile([P, SC * NG], BF16, tag="x0c")
                    nc.sync.dma_start(
                        out=x0c[:, :Ssc * 512],
                        in_=x0T[:, grp * NG: grp * NG + Ssc * 512])
                    el2c = el2p.tile([P, SC * T * ML2], BF16, tag="el2c")
                    nc.sync.dma_start(
                        out=el2c[:, :Tsc * ML2],
                        in_=embL2[grp // SC][:, :Tsc * ML2])
                goff = (grp % SC) * NG          # node offset inside chunk
                toff = (grp % SC) * T           # tile offset inside chunk

                if grp % OB == 0:
                    scst = stp.tile([1, OB * NG], F32, tag="scst")
                    Sst = stp.tile([W, OB * ML2], F32, tag="Sst")
                boff = grp % OB

                # ---- scalar (MLP) branch, 512 nodes at a time ----
                for s in range(Sr):
                    nsl = slice(goff + s * 512, goff + (s + 1) * 512)
                    h1p = ph1p.tile([P, 512], F32, tag="h1p")
                    nc.tensor.matmul(h1p[:], w1s[:], x0c[:, nsl],
                                     start=True, stop=True)
                    h1s = hp.tile([P, 512], BF16, tag="h1s")
                    nc.scalar.activation(h1s[:], h1p[:], silu, bias=b1s[:])
                    h2p = ph2p.tile([P, 512], F32, tag="h2p")
                    nc.tensor.matmul(h2p[:], w2s[:], h1s[:],
                                     start=True, stop=True)
                    h2s = hp.tile([P, 512], BF16, tag="h2s")
                    nc.scalar.activation(h2s[:], h2p[:], silu, bias=b2s[:])
                    scp = pscp.tile([1, 512], F32, tag="scp")
                    nc.tensor.matmul(scp[:], w3s[:], h2s[:],
                                     start=True, stop=True)
                    nc.vector.tensor_copy(
                        out=scst[:, boff * NG + s * 512:
                                 boff * NG + (s + 1) * 512],
                        in_=scp[:])

                # ---- l=2 branch with fused segment sum ----
                # indicator rows for all tiles of the group in one DVE op
                A8 = apool.tile([P, T * W], BF16, tag="A8")
                nc.vector.tensor_tensor(
                    out=A8[:].rearrange("p (t w) -> p t w", t=T, w=W),
                    in0=iotas[:].unsqueeze(1).to_broadcast([P, T, W]),
                    in1=lgids[:, grp * T: (grp + 1) * T]
                        .unsqueeze(2).to_broadcast([P, T, W]),
                    op=eq)
                pSa = pSp.tile([W, 320], F32, tag="pSa")
                pSb = pSp.tile([W, 320], F32, tag="pSb")
                for t in range(Tr):
                    At = A8[:, t * W:(t + 1) * W]
                    base = (toff + t) * ML2
                    nc.tensor.matmul(pSa[:], At, el2c[:, base: base + 320],
                                     start=(t == 0), stop=(t == Tr - 1))
                    nc.tensor.matmul(pSb[:], At,
                                     el2c[:, base + 320: base + ML2],
                                     start=(t == 0), stop=(t == Tr - 1))
                nc.vector.tensor_copy(out=Sst[:, boff * ML2: boff * ML2 + 320],
                                      in_=pSa[:])
                nc.vector.tensor_copy(out=Sst[:, boff * ML2 + 320:
                                              (boff + 1) * ML2],
                                      in_=pSb[:])

                if grp % OB == OB - 1 or grp == n_groups - 1:
                    ob = grp // OB
                    nc.scalar.dma_start(out=scal[ob: ob + 1, :], in_=scst[:])
                    nc.scalar.dma_start(out=S_out[ob], in_=Sst[:])

    nc.compile()
    _BUILD_CACHE[key] = nc
    return nc


def _next_pow2(x):
    p = 8
    while p < x:
        p *= 2
    return p


def _host_reference(node_embedding, W1, b1, W2, b2, W3, b3, w_l2, batch,
                    natoms):
    """Pure-numpy fallback (only used for pathological graph layouts)."""
    G = natoms.shape[0]
    inv = 1.0 / natoms.astype(np.float32)
    x = node_embedding[:, 0, :]
    h = x @ W1.T + b1
    h = h / (1.0 + np.exp(-h))
    h = h @ W2.T + b2
    h = h / (1.0 + np.exp(-h))
    ns = (h @ W3.T + b3)[:, 0]
    iso = np.bincount(batch, weights=ns, minlength=G).astype(np.float32) * inv
    nl2 = np.einsum("nmc,c->nm", node_embedding[:, 4:9, :], w_l2[0])
    aniso = np.zeros((G, 5), np.float32)
    np.add.at(aniso, batch, nl2)
    aniso *= inv[:, None]
    dec = np.concatenate([iso[:, None], np.zeros((G, 3), np.float32), aniso],
                         axis=1)
    return (dec @ _CG).reshape(-1, 3, 3).astype(np.float32)


def kernel(node_embedding, W1, b1, W2, b2, W3, b3, w_l2, batch, natoms):
    node_embedding = np.asarray(node_embedding, dtype=np.float32)
    W1 = np.asarray(W1, dtype=np.float32)
    b1 = np.asarray(b1, dtype=np.float32)
    W2 = np.asarray(W2, dtype=np.float32)
    b2 = np.asarray(b2, dtype=np.float32)
    W3 = np.asarray(W3, dtype=np.float32)
    b3 = np.asarray(b3, dtype=np.float32)
    w_l2 = np.asarray(w_l2, dtype=np.float32)
    batch = np.asarray(batch).astype(np.int64)
    natoms_in = np.asarray(natoms)

    N = node_embedding.shape[0]
    G = natoms_in.shape[0]
    n_sh = (N + N_CORES - 1) // N_CORES
    n_groups = (n_sh + NG - 1) // NG
    n_pad = n_groups * NG
    n_sc = (n_groups + SC - 1) // SC

    # per-core shard ranges and group graph bases
    shards = []
    W_need = 8
    for c in range(N_CORES):
        n0 = min(c * n_sh, N)
        n1 = min(n0 + n_sh, N)
        b = batch[n0:n1]
        nreal = n1 - n0
        gbase = np.zeros(n_groups, np.int64)
        for grp in range(n_groups):
            lo = grp * NG
            hi = min(lo + NG, nreal)
            if lo < nreal:
                gbase[grp] = b[lo]
                span = int(b[hi - 1] - b[lo] + 1)
                W_need = max(W_need, span)
        shards.append((n0, n1, b, gbase))
    W = _next_pow2(W_need)
    if W > P or not np.all(batch[:-1] <= batch[1:]):
        return _host_reference(node_embedding, W1, b1, W2, b2, W3, b3,
                               w_l2, batch, natoms_in)

    nc = _build(n_pad, n_groups, W, n_sh)

    w1t = np.ascontiguousarray(W1.T).astype(WIRE)
    w2t = np.ascontiguousarray(W2.T).astype(WIRE)
    w3t = np.ascontiguousarray(W3.T).astype(WIRE)
    b1c = np.ascontiguousarray(b1[:, None])
    b2c = np.ascontiguousarray(b2[:, None])
    iota_c = np.ascontiguousarray(
        np.tile(np.arange(W, dtype=np.float32), (P, 1)))

    in_maps = []
    for c in range(N_CORES):
        n0, n1, b, gbase = shards[c]
        nreal = n1 - n0
        x0T = np.zeros((P, n_pad), WIRE)
        x0T[:, :nreal] = node_embedding[n0:n1, 0, :].T.astype(WIRE)
        # pre-tiled l=2 data: [sc, p, (tile, m)] so each partition's chunk
        # read is one contiguous run
        el2 = np.zeros((n_sc * SC * NG, ML2), WIRE)
        el2[:nreal] = node_embedding[n0:n1, 4:9, :].reshape(nreal, ML2) \
            .astype(WIRE)
        el2 = np.ascontiguousarray(
            el2.reshape(n_sc, SC * T, P, ML2).transpose(0, 2, 1, 3)
               .reshape(n_sc, P, SC * T * ML2))
        lg = np.full(n_pad, -1.0, np.float32)
        lg[:nreal] = (b - np.repeat(gbase, NG)[:nreal]).astype(np.float32)
        lg_t = np.ascontiguousarray(
            lg.reshape(n_groups, T, P).transpose(2, 0, 1)
              .reshape(P, n_groups * T))
        in_maps.append({
            "x0T": x0T, "embL2": el2, "lgid": lg_t, "iota_in": iota_c,
            "w1t": w1t, "w2t": w2t, "w3t": w3t, "b1c": b1c, "b2c": b2c,
        })

    res = bass_utils.run_bass_kernel_spmd(nc, in_maps,
                                          core_ids=list(range(N_CORES)))

    # ---- host epilogue ----
    inv = (1.0 / natoms_in.astype(np.float32)).astype(np.float32)
    node_scalar = np.empty(N, np.float32)
    Sfull = np.zeros((G + P, ML2), np.float32)
    for c in range(N_CORES):
        n0, n1, _, gbase = shards[c]
        nreal = n1 - n0
        sc = res.results[c]["scal"].reshape(-1)[:nreal]
        node_scalar[n0:n1] = sc
        Sc = res.results[c]["S_out"]        # (n_ob, W, OB*640)
        for grp in range(n_groups):
            if grp * NG < nreal:
                gb = int(gbase[grp])
                j = grp % OB
                Sfull[gb:gb + W] += Sc[grp // OB][:, j * ML2:(j + 1) * ML2]
    iso = np.bincount(batch, weights=node_scalar + b3[0], minlength=G)
    iso = iso.astype(np.float32) * inv
    aniso = (Sfull[:G].reshape(G, 5, P) @ w_l2[0]) * inv[:, None]
    dec = np.concatenate([iso[:, None], np.zeros((G, 3), np.float32), aniso],
                         axis=1)
    return (dec @ _CG).reshape(-1, 3, 3).astype(np.float32)


# revision 9
# speedup vs baseline: 1.8356x; 1.0110x over previous
"""CartBasisStressHead kernel for Trainium2 (8 NeuronCores, SPMD data-parallel).

Strategy
--------
Only 6 of the 9 m-rows of node_embedding are used: row 0 feeds a SiLU MLP
(per-node scalar), rows 4:9 feed a per-channel contraction (l=2 branch).
Nodes are sharded contiguously across 8 cores (graphs may straddle shard
boundaries; segment sums are linear, so host adds the partials).

Per core the device computes, in 1024-node groups:
  * scalar branch: h = silu(W1 @ x0T + b1); h = silu(W2 @ h + b2);
    s = W3 @ h  -> per-node scalars (feature-major layout, x0 transposed
    on host so the contraction dim lands on SBUF partitions).
  * l=2 branch fused with the segment sum: a 0/1 indicator matrix
    A[node, local_graph] (built on the vector engine from iota == local
    graph id) is the stationary matmul operand, so
    S[g_local, (m,c)] = sum_n A[n,g] * emb[n, (m,c)] accumulates in PSUM
    across the group's 8 node-tiles. Data streams through the PE once.

Matmul data travels in bf16 (fp32 accumulation in PSUM). Inputs are loaded
in 2-group (~2.6 MB) chunks, outputs staged in SBUF over 4 groups before a
single store, so nearly all DMA time is large transfers.

Host epilogue: bincount segment-sum of per-node scalars, scatter-add of
per-group S partials, contraction with w_l2, and the tiny (G,9)@(9,9)
change-of-basis.
"""

import sys

if "/opt/trn_rl_repo" not in sys.path:
    sys.path.insert(0, "/opt/trn_rl_repo")

import numpy as np
import ml_dtypes

import concourse.bacc as bacc
import concourse.tile as tile
from concourse import mybir
from concourse import bass_utils

_S2 = 2.0 ** -0.5
_S3 = 3.0 ** -0.5
_S6 = 6.0 ** -0.5
_CG = np.array([
    [_S3, 0, 0, 0, _S3, 0, 0, 0, _S3],
    [0, 0, 0, 0, 0, _S2, 0, -_S2, 0],
    [0, 0, -_S2, 0, 0, 0, _S2, 0, 0],
    [0, _S2, 0, -_S2, 0, 0, 0, 0, 0],
    [0, 0, _S2, 0, 0, 0, _S2, 0, 0],
    [0, 0, 0, 0, 0, _S2, 0, _S2, 0],
    [-_S6, 0, 0, 0, 2 * _S6, 0, 0, 0, -_S6],
    [0, _S2, 0, _S2, 0, 0, 0, 0, 0],
    [-_S2, 0, 0, 0, 0, 0, 0, 0, _S2],
], dtype=np.float32)  # (9, 9)

N_CORES = 8
P = 128          # SBUF partitions
NG = 1024        # nodes per group (one PSUM accumulation span)
T = NG // P      # node-tiles per group
ML2 = 5 * P      # 640 values of l=2 data per node
SC = 2           # groups per input superchunk
OB = 4           # groups per output staging batch

F32 = mybir.dt.float32
BF16 = mybir.dt.bfloat16
WIRE = ml_dtypes.bfloat16

_BUILD_CACHE = {}


def _build(n_pad, n_groups, W, n_real):
    key = (n_pad, n_groups, W, n_real)
    if key in _BUILD_CACHE:
        return _BUILD_CACHE[key]

    n_sc = (n_groups + SC - 1) // SC
    n_ob = (n_groups + OB - 1) // OB

    nc = bacc.Bacc("TRN2", target_bir_lowering=False, debug=False,
                   num_devices=N_CORES)

    x0T = nc.dram_tensor("x0T", (P, n_pad), BF16, kind="ExternalInput").ap()
    # host pre-tiled: [sc, p, sc_tile*640] fully contiguous per partition
    embL2 = nc.dram_tensor("embL2", (n_sc, P, SC * T * ML2), BF16,
                           kind="ExternalInput").ap()
    lgid = nc.dram_tensor("lgid", (P, n_groups * T), F32,
                          kind="ExternalInput").ap()
    iota_in = nc.dram_tensor("iota_in", (P, W), F32, kind="ExternalInput").ap()
    w1t = nc.dram_tensor("w1t", (P, P), BF16, kind="ExternalInput").ap()
    w2t = nc.dram_tensor("w2t", (P, P), BF16, kind="ExternalInput").ap()
    w3t = nc.dram_tensor("w3t", (P, 1), BF16, kind="ExternalInput").ap()
    b1 = nc.dram_tensor("b1c", (P, 1), F32, kind="ExternalInput").ap()
    b2 = nc.dram_tensor("b2c", (P, 1), F32, kind="ExternalInput").ap()
    scal = nc.dram_tensor("scal", (n_ob, OB * NG), F32,
                          kind="ExternalOutput").ap()
    S_out = nc.dram_tensor("S_out", (n_ob, W, OB * ML2), F32,
                           kind="ExternalOutput").ap()

    silu = mybir.ActivationFunctionType.Silu
    eq = mybir.AluOpType.is_equal

    with tile.TileContext(nc) as tc:
        with (
            tc.tile_pool(name="const", bufs=1) as cpool,
            tc.tile_pool(name="x0p", bufs=3) as x0p,
            tc.tile_pool(name="el2p", bufs=3) as el2p,
            tc.tile_pool(name="hp", bufs=3) as hp,
            tc.tile_pool(name="apl", bufs=2) as apool,
            tc.tile_pool(name="stp", bufs=2) as stp,
            tc.tile_pool(name="ph1", bufs=2, space="PSUM") as ph1p,
            tc.tile_pool(name="ph2", bufs=1, space="PSUM") as ph2p,
            tc.tile_pool(name="psc", bufs=1, space="PSUM") as pscp,
            tc.tile_pool(name="pS", bufs=2, space="PSUM") as pSp,
        ):
            w1s = cpool.tile([P, P], BF16)
            w2s = cpool.tile([P, P], BF16)
            w3s = cpool.tile([P, 1], BF16)
            b1s = cpool.tile([P, 1], F32)
            b2s = cpool.tile([P, 1], F32)
            iotas = cpool.tile([P, W], F32)
            lgids = cpool.tile([P, n_groups * T], F32)
            nc.sync.dma_start(out=w1s[:], in_=w1t)
            nc.sync.dma_start(out=w2s[:], in_=w2t)
            nc.sync.dma_start(out=w3s[:], in_=w3t)
            nc.sync.dma_start(out=b1s[:], in_=b1)
            nc.sync.dma_start(out=b2s[:], in_=b2)
            nc.sync.dma_start(out=iotas[:], in_=iota_in)
            nc.sync.dma_start(out=lgids[:], in_=lgid)

            x0c = el2c = None
            scst = Sst = None
            for grp in range(n_groups):
                grp_real = min(NG, n_real - grp * NG)
                Tr = (grp_real + P - 1) // P
                Sr = (grp_real + 511) // 512

                if grp % SC == 0:
                    # tiles with real nodes in this superchunk
                    sc_real = min(SC * NG, n_real - grp * NG)
                    Tsc = (sc_real + P - 1) // P
                    Ssc = (sc_real + 511) // 512
                    x0c = x0p.tile([P, SC * NG], BF16, tag="x0c")
                    nc.sync.dma_start(
                        out=x0c[:, :Ssc * 512],
                        in_=x0T[:, grp * NG: grp * NG + Ssc * 512])
                    el2c = el2p.tile([P, SC * T * ML2], BF16, tag="el2c")
                    nc.sync.dma_start(
                        out=el2c[:, :Tsc * ML2],
                        in_=embL2[grp // SC][:, :Tsc * ML2])
                goff = (grp % SC) * NG          # node offset inside chunk
                toff = (grp % SC) * T           # tile offset inside chunk

                if grp % OB == 0:
                    scst = stp.tile([1, OB * NG], F32, tag="scst")
                    Sst = stp.tile([W, OB * ML2], F32, tag="Sst")
                boff = grp % OB

                # ---- scalar (MLP) branch, 512 nodes at a time ----
                for s in range(Sr):
                    nsl = slice(goff + s * 512, goff + (s + 1) * 512)
                    h1p = ph1p.tile([P, 512], F32, tag="h1p")
                    nc.tensor.matmul(h1p[:], w1s[:], x0c[:, nsl],
                                     start=True, stop=True)
                    h1s = hp.tile([P, 512], BF16, tag="h1s")
                    nc.scalar.activation(h1s[:], h1p[:], silu, bias=b1s[:])
                    h2p = ph2p.tile([P, 512], F32, tag="h2p")
                    nc.tensor.matmul(h2p[:], w2s[:], h1s[:],
                                     start=True, stop=True)
                    h2s = hp.tile([P, 512], BF16, tag="h2s")
                    nc.scalar.activation(h2s[:], h2p[:], silu, bias=b2s[:])
                    scp = pscp.tile([1, 512], F32, tag="scp")
                    nc.tensor.matmul(scp[:], w3s[:], h2s[:],
                                     start=True, stop=True)
                    nc.vector.tensor_copy(
                        out=scst[:, boff * NG + s * 512:
                                 boff * NG + (s + 1) * 512],
                        in_=scp[:])

                # ---- l=2 branch with fused segment sum ----
                # indicator rows for all tiles of the group in one DVE op
                A8 = apool.tile([P, T * W], BF16, tag="A8")
                nc.vector.tensor_tensor(
                    out=A8[:].rearrange("p (t w) -> p t w", t=T, w=W),
                    in0=iotas[:].unsqueeze(1).to_broadcast([P, T, W]),
                    in1=lgids[:, grp * T: (grp + 1) * T]
                        .unsqueeze(2).to_broadcast([P, T, W]),
                    op=eq)
                pSa = pSp.tile([W, 320], F32, tag="pSa")
                pSb = pSp.tile([W, 320], F32, tag="pSb")
                for t in range(Tr):
                    At = A8[:, t * W:(t + 1) * W]
                    base = (toff + t) * ML2
                    nc.tensor.matmul(pSa[:], At, el2c[:, base: base + 320],
                                     start=(t == 0), stop=(t == Tr - 1))
                    nc.tensor.matmul(pSb[:], At,
                                     el2c[:, base + 320: base + ML2],
                                     start=(t == 0), stop=(t == Tr - 1))
                nc.vector.tensor_copy(out=Sst[:, boff * ML2: boff * ML2 + 320],
                                      in_=pSa[:])
                nc.vector.tensor_copy(out=Sst[:, boff * ML2 + 320:
                                              (boff + 1) * ML2],
                                      in_=pSb[:])

                if grp % OB == OB - 1 or grp == n_groups - 1:
                    ob = grp // OB
                    nc.scalar.dma_start(out=scal[ob: ob + 1, :], in_=scst[:])
                    nc.scalar.dma_start(out=S_out[ob], in_=Sst[:])

    nc.compile()
    _BUILD_CACHE[key] = nc
    return nc


def _next_pow2(x):
    p = 8
    while p < x:
        p *= 2
    return p


def _host_reference(node_embedding, W1, b1, W2, b2, W3, b3, w_l2, batch,
                    natoms):
    """Pure-numpy fallback (only used for pathological graph layouts)."""
    G = natoms.shape[0]
    inv = 1.0 / natoms.astype(np.float32)
    x = node_embedding[:, 0, :]
    h = x @ W1.T + b1
    h = h / (1.0 + np.exp(-h))
    h = h @ W2.T + b2
    h = h / (1.0 + np.exp(-h))
    ns = (h @ W3.T + b3)[:, 0]
    iso = np.bincount(batch, weights=ns, minlength=G).astype(np.float32) * inv
    nl2 = np.einsum("nmc,c->nm", node_embedding[:, 4:9, :], w_l2[0])
    aniso = np.zeros((G, 5), np.float32)
    np.add.at(aniso, batch, nl2)
    aniso *= inv[:, None]
    dec = np.concatenate([iso[:, None], np.zeros((G, 3), np.float32), aniso],
                         axis=1)
    return (dec @ _CG).reshape(-1, 3, 3).astype(np.float32)


def kernel(node_embedding, W1, b1, W2, b2, W3, b3, w_l2, batch, natoms):
    node_embedding = np.asarray(node_embedding, dtype=np.float32)
    W1 = np.asarray(W1, dtype=np.float32)
    b1 = np.asarray(b1, dtype=np.float32)
    W2 = np.asarray(W2, dtype=np.float32)
    b2 = np.asarray(b2, dtype=np.float32)
    W3 = np.asarray(W3, dtype=np.float32)
    b3 = np.asarray(b3, dtype=np.float32)
    w_l2 = np.asarray(w_l2, dtype=np.float32)
    batch = np.asarray(batch).astype(np.int64)
    natoms_in = np.asarray(natoms)

    N = node_embedding.shape[0]
    G = natoms_in.shape[0]
    n_sh = (N + N_CORES - 1) // N_CORES
    n_groups = (n_sh + NG - 1) // NG
    n_pad = n_groups * NG
    n_sc = (n_groups + SC - 1) // SC

    # per-core shard ranges and group graph bases
    shards = []
    W_need = 8
    for c in range(N_CORES):
        n0 = min(c * n_sh, N)
        n1 = min(n0 + n_sh, N)
        b = batch[n0:n1]
        nreal = n1 - n0
        gbase = np.zeros(n_groups, np.int64)
        for grp in range(n_groups):
            lo = grp * NG
            hi = min(lo + NG, nreal)
            if lo < nreal:
                gbase[grp] = b[lo]
                span = int(b[hi - 1] - b[lo] + 1)
                W_need = max(W_need, span)
        shards.append((n0, n1, b, gbase))
    W = _next_pow2(W_need)
    if W > P or not np.all(batch[:-1] <= batch[1:]):
        return _host_reference(node_embedding, W1, b1, W2, b2, W3, b3,
                               w_l2, batch, natoms_in)

    nc = _build(n_pad, n_groups, W, n_sh)

    w1t = np.ascontiguousarray(W1.T).astype(WIRE)
    w2t = np.ascontiguousarray(W2.T).astype(WIRE)
    w3t = np.ascontiguousarray(W3.T).astype(WIRE)
    b1c = np.ascontiguousarray(b1[:, None])
    b2c = np.ascontiguousarray(b2[:, None])
    iota_c = np.ascontiguousarray(
        np.tile(np.arange(W, dtype=np.float32), (P, 1)))

    in_maps = []
    for c in range(N_CORES):
        n0, n1, b, gbase = shards[c]
        nreal = n1 - n0
        x0T = np.zeros((P, n_pad), WIRE)
        x0T[:, :nreal] = node_embedding[n0:n1, 0, :].T.astype(WIRE)
        # pre-tiled l=2 data: [sc, p, (tile, m)] so each partition's chunk
        # read is one contiguous run
        el2 = np.zeros((n_sc * SC * NG, ML2), WIRE)
        el2[:nreal] = node_embedding[n0:n1, 4:9, :].reshape(nreal, ML2) \
            .astype(WIRE)
        el2 = np.ascontiguousarray(
            el2.reshape(n_sc, SC * T, P, ML2).transpose(0, 2, 1, 3)
               .reshape(n_sc, P, SC * T * ML2))
        lg = np.full(n_pad, -1.0, np.float32)
        lg[:nreal] = (b - np.repeat(gbase, NG)[:nreal]).astype(np.float32)
        lg_t = np.ascontiguousarray(
            lg.reshape(n_groups, T, P).transpose(2, 0, 1)
              .reshape(P, n_groups * T))
        in_maps.append({
            "x0T": x0T, "embL2": el2, "lgid": lg_t, "iota_in": iota_c,
            "w1t": w1t, "w2t": w2t, "w3t": w3t, "b1c": b1c, "b2c": b2c,
        })

    res = bass_utils.run_bass_kernel_spmd(nc, in_maps,
                                          core_ids=list(range(N_CORES)))

    # ---- host epilogue ----
    inv = (1.0 / natoms_in.astype(np.float32)).astype(np.float32)
    node_scalar = np.empty(N, np.float32)
    Sfull = np.zeros((G + P, ML2), np.float32)
    for c in range(N_CORES):
        n0, n1, _, gbase = shards[c]
        nreal = n1 - n0
        sc = res.results[c]["scal"].reshape(-1)[:nreal]
        node_scalar[n0:n1] = sc
        Sc = res.results[c]["S_out"]        # (n_ob, W, OB*640)
        for grp in range(n_groups):
            if grp * NG < nreal:
                gb = int(gbase[grp])
                j = grp % OB
                Sfull[gb:gb + W] += Sc[grp // OB][:, j * ML2:(j + 1) * ML2]
    iso = np.bincount(batch, weights=node_scalar + b3[0], minlength=G)
    iso = iso.astype(np.float32) * inv
    aniso = (Sfull[:G].reshape(G, 5, P) @ w_l2[0]) * inv[:, None]
    dec = np.concatenate([iso[:, None], np.zeros((G, 3), np.float32), aniso],
                         axis=1)
    return (dec @ _CG).reshape(-1, 3, 3).astype(np.float32)
